# revision 37
# baseline (speedup 1.0000x reference)
"""Trainium2 Bass kernel for nn_BinaryMNModel (binary Markov-network clique scoring).

Math: for each batch row b,
    ll[b] = sum_c sum_j f[c,j] * prod_s ( bc[j,s] ? x[b,vars[c,s]] : 1-x[b,vars[c,s]] )

We re-express each clique's factor table in the multilinear monomial basis
(a 8x8 +-1 transform of the 8 factor entries):
    score[c,b] = g0[c] + g1[c]*a0 + g2[c]*a1 + g3[c]*a2
               + g4[c]*a0*a1 + g5[c]*a0*a2 + g6[c]*a1*a2 + g7[c]*a0*a1*a2
with a_s = x[b, vars[c,s]].  Summing over cliques:
  - the constant term becomes one host-side scalar,
  - the linear terms fold into a V-length weight vector w (host scatter-add),
    so sum_c(linear) = x @ w  (done on-device, V-sharded across cores),
  - only the 4 quadratic/cubic monomials need the gathered values.

Sharding: cliques are sharded across the 8 cores (2500 each); the x@w matvec
is V-sharded.  Each core returns a partial [256] vector; host sums them.

Per core on device:
  - dma_gather (GPSIMD SWDGE) pulls rows of x^T [V, B] from DRAM for each
    clique slot into a_s tiles [128 cliques (part), 20 chunks, 256 b (free)].
    Descriptor generation is the serial cost (~9 ns/row on a Q7 core pair),
    so gathers are spread over all 4 SWDGE queues (= 4 Q7 core pairs) with
    greedy load balancing, and group sizes ramp up so the pipeline fills fast.
  - DVE computes the 4 products p01, p02, p12, p012=p01*a2 per group.
  - PE reduces each weighted monomial over cliques via fp32 matmuls with the
    g column as stationary [128,1], running concurrently in the PE's 128x32
    column-groups 0/32/64 (col-group 3 is a known TRN2 HW bug), accumulating
    into psum rows 0/32/64; the x@w matvec rides the row-0 chain.  The three
    rows are summed on DVE at the end.
"""

import os
import sys

import numpy as np

# ---------------------------------------------------------------- constants
B = 256
V = 5000
C = 20000
S = 3
NCOMB = 8
N_CORES = 8

C_SHARD = C // N_CORES          # 2500 cliques per core
CHUNKS = 20                     # 2560 = 20 * 128
C_PAD = CHUNKS * 128            # padded cliques per core
# ramped group sizes (in 128-clique chunks): small first groups so the first
# products start as early as possible, hiding the DMA-sem receipt latency
GROUP_CHUNKS = [1, 2, 3, 3, 3, 3, 3, 2]
assert sum(GROUP_CHUNKS) == CHUNKS
IDX_COLS = C_PAD // 16          # 160 columns in the wrapped idx layout

V_SHARD = V // N_CORES          # 625
V_CHUNKS = 5                    # padded to 640 = 5 * 128

# aux layout (f32): [coef 4*CHUNKS | xv V_CHUNKS*B | wv V_CHUNKS]
COEF_OFF = 0
XV_OFF = 4 * CHUNKS
WV_OFF = XV_OFF + V_CHUNKS * B
AUX_COLS = WV_OFF + V_CHUNKS

_PROGRAM = None  # compiled program cache: (nc, out_name)

# matmul input dtype for the weighted clique reductions: "f32" (exact, 4
# cyc/row) or "f32r" (single-pass, 1 cyc/row at N>=256, reduced precision)
MM_DTYPE = os.environ.get("K_MM_DTYPE", "f32")
N_QUEUES = int(os.environ.get("K_NQ", "4"))


def _build_program():
    import concourse.bass as bass
    import concourse.mybir as mybir
    from concourse import bacc, tile

    f32 = mybir.dt.float32
    i16 = mybir.dt.int16
    MULT = mybir.AluOpType.mult
    ADD = mybir.AluOpType.add

    nc = bacc.Bacc(
        "TRN2",
        target_bir_lowering=False,
        debug=False,
        enable_asserts=False,
        num_devices=N_CORES,
        num_swdge_queues=max(N_QUEUES, 1),
    )

    xt_d = nc.dram_tensor("xt", [V, B], f32, kind="ExternalInput")
    idx_d = nc.dram_tensor("idx", [128, 3 * IDX_COLS], i16, kind="ExternalInput")
    aux_d = nc.dram_tensor("aux", [128, AUX_COLS], f32, kind="ExternalInput")
    out_d = nc.dram_tensor("out", [1, B], f32, kind="ExternalOutput")

    with tile.TileContext(nc) as tc:
        with (
            tc.tile_pool(name="persist", bufs=1) as pp,
            tc.tile_pool(name="prod", bufs=5) as prodp,
            tc.tile_pool(name="ps", bufs=1, space="PSUM") as psp,
        ):
            idx_t = pp.tile([128, 3 * IDX_COLS], i16, tag="idx")
            aux_t = pp.tile([128, AUX_COLS], f32, tag="aux")
            a_t = [
                pp.tile([128, CHUNKS, B], f32, tag=f"a{s}", name=f"a{s}")
                for s in range(S)
            ]
            out_sb = pp.tile([1, B], f32, tag="out_sb")
            tmp_s = pp.tile([1, B], f32, tag="tmp_s")
            tmp_t = pp.tile([1, B], f32, tag="tmp_t")
            psum_t = psp.tile([128, B], f32, tag="psum")

            nc.sync.dma_start(idx_t[:], idx_d[:])
            nc.sync.dma_start(aux_t[:], aux_d[:])

            # PE accumulation: weighted clique reductions run concurrently in
            # the PE's 128x32 column-groups (col-group 3 is a known HW bug, so
            # only groups 0/32/64 are used).  m01 -> row 0, m02 -> row 32,
            # m12 -> row 64; m012 is split between rows 32 and 64 to balance.
            row_started = set()

            def mm(mono, chunk, moving_ap, last=False):
                coef_col = mono * CHUNKS + chunk
                lhs = aux_t[:, COEF_OFF + coef_col : COEF_OFF + coef_col + 1]
                if mono < 3:
                    row = 32 * mono
                else:
                    row = 32 if chunk < CHUNKS // 2 else 64
                nc.tensor.matmul(
                    psum_t[row : row + 1, :],
                    lhs,
                    moving_ap,
                    start=(row not in row_started),
                    stop=last,
                    tile_position=(0, row),
                )
                row_started.add(row)

            # all gathers first: they are gpsimd's only work and pace the rest.
            # queue assignment must be pure round-robin: Tile's 8 DMASW sem
            # lanes are assigned round-robin per call and each lane is locked
            # to one SWDGE queue, so queue must equal call_index % N_QUEUES.
            gi = 0
            c_off = [0]
            for gc in GROUP_CHUNKS:
                c_off.append(c_off[-1] + gc)
            for g, gc in enumerate(GROUP_CHUNKS):
                c0, c1 = c_off[g], c_off[g + 1]
                n_idx = gc * 128
                for s in range(S):
                    q = gi % N_QUEUES
                    gi += 1
                    nc.gpsimd.dma_gather(
                        a_t[s][:, c0:c1, :],
                        xt_d[:],
                        idx_t[:, s * IDX_COLS + c0 * 8 : s * IDX_COLS + c1 * 8],
                        n_idx,
                        n_idx,
                        B,
                        queue_num=q,
                    )

            # linear terms: x @ w on the PE row-0 chain (data ready early)
            for j in range(V_CHUNKS):
                nc.tensor.matmul(
                    psum_t[0:1, :],
                    aux_t[:, WV_OFF + j : WV_OFF + j + 1],
                    aux_t[:, XV_OFF + j * B : XV_OFF + (j + 1) * B],
                    start=(j == 0),
                    stop=False,
                    tile_position=(0, 0),
                )
            row_started.add(0)

            for g, gc in enumerate(GROUP_CHUNKS):
                c0, c1 = c_off[g], c_off[g + 1]
                a0g = a_t[0][:, c0:c1, :]
                a1g = a_t[1][:, c0:c1, :]
                a2g = a_t[2][:, c0:c1, :]
                p01 = prodp.tile([128, gc, B], f32, tag="p01", name="p01")
                p02 = prodp.tile([128, gc, B], f32, tag="p02", name="p02")
                p12 = prodp.tile([128, gc, B], f32, tag="p12", name="p12")
                p012 = prodp.tile([128, gc, B], f32, tag="p012", name="p012")
                nc.vector.tensor_tensor(p01[:], a0g, a1g, MULT)
                nc.vector.tensor_tensor(p02[:], a0g, a2g, MULT)
                nc.vector.tensor_tensor(p12[:], a1g, a2g, MULT)
                nc.vector.tensor_tensor(p012[:], p01[:], a2g, MULT)
                last = g == len(GROUP_CHUNKS) - 1
                for c in range(gc):
                    ci = c0 + c
                    lc = last and c == gc - 1
                    mm(0, ci, p01[:, c, :], last=lc)   # row 0 ends with m01
                    mm(1, ci, p02[:, c, :], last=lc)   # row 32 ends with m02
                    mm(2, ci, p12[:, c, :])
                    mm(3, ci, p012[:, c, :], last=lc)  # row 64 ends with m012

            # combine the 3 chain rows (0, 32, 64) into the output
            # (DVE may read at most one PSUM operand per instruction)
            nc.vector.tensor_copy(tmp_s[:], psum_t[0:1, :])
            nc.vector.tensor_add(tmp_t[:], tmp_s[:], psum_t[32:33, :])
            nc.vector.tensor_add(out_sb[:], tmp_t[:], psum_t[64:65, :])
            nc.sync.dma_start(out_d[:], out_sb[:])

    nc.compile()
    return nc, out_d.name


def get_program():
    global _PROGRAM
    if _PROGRAM is None:
        _PROGRAM = _build_program()
    return _PROGRAM


# ---------------------------------------------------------------- host prep
def _monomial_transform(all_factors: np.ndarray) -> np.ndarray:
    """g[c,t] such that score[c,b] = sum_t g[c,t] * prod_{s: bit (S-1-s) of t} a_s."""
    M = np.zeros((NCOMB, NCOMB), dtype=np.float64)
    for t in range(NCOMB):
        for j in range(NCOMB):
            if j & ~t:
                continue
            M[t, j] = (-1.0) ** bin(t & ~j).count("1")
    return all_factors.astype(np.float64) @ M.T


def _wrap_idx(idx: np.ndarray) -> np.ndarray:
    """[C_PAD] int -> [128, IDX_COLS] int16 dma_gather layout (idx i at
    partition i%16, col i//16, replicated across the 8 q7 cores)."""
    w = idx.reshape(IDX_COLS, 16).T.astype(np.int16)  # [16, IDX_COLS]
    return np.tile(w, (8, 1))


def _chunk_layout(v: np.ndarray) -> np.ndarray:
    """[C_PAD] -> [128, CHUNKS]: element i at partition i%128, col i//128."""
    return np.ascontiguousarray(v.reshape(CHUNKS, 128).T)


def prepare_inputs(x, all_vars, all_factors):
    x = np.asarray(x, dtype=np.float32)
    all_vars = np.asarray(all_vars)
    all_factors = np.asarray(all_factors, dtype=np.float32)

    xt = np.ascontiguousarray(x.T)  # [V, B] f32

    g = _monomial_transform(all_factors)  # [C, 8] f64
    bit = [1 << (S - 1 - s) for s in range(S)]
    t01, t02, t12 = bit[0] | bit[1], bit[0] | bit[2], bit[1] | bit[2]
    t012 = bit[0] | bit[1] | bit[2]

    const0 = float(g[:, 0].sum())
    w = np.zeros(V, dtype=np.float64)
    for s in range(S):
        np.add.at(w, all_vars[:, s], g[:, bit[s]])
    w = w.astype(np.float32)
    g32 = g.astype(np.float32)

    in_maps = []
    for k in range(N_CORES):
        sl = slice(k * C_SHARD, (k + 1) * C_SHARD)
        pad = C_PAD - C_SHARD

        idx_parts = []
        for s in range(S):
            ii = np.concatenate([all_vars[sl, s], np.zeros(pad, np.int64)])
            idx_parts.append(_wrap_idx(ii))
        idx_arr = np.ascontiguousarray(np.concatenate(idx_parts, axis=1))

        coef_cols = []
        for t in (t01, t02, t12, t012):
            gg = np.concatenate([g32[sl, t], np.zeros(pad, np.float32)])
            coef_cols.append(_chunk_layout(gg))
        coef_arr = np.concatenate(coef_cols, axis=1)  # [128, 4*CHUNKS]

        vs = slice(k * V_SHARD, (k + 1) * V_SHARD)
        vpad = V_CHUNKS * 128 - V_SHARD
        xv = np.concatenate([xt[vs], np.zeros((vpad, B), np.float32)])
        xv = xv.reshape(V_CHUNKS, 128, B).transpose(1, 0, 2).reshape(128, V_CHUNKS * B)
        wv = np.concatenate([w[vs], np.zeros(vpad, np.float32)])
        wv = np.ascontiguousarray(wv.reshape(V_CHUNKS, 128).T)

        aux = np.ascontiguousarray(
            np.concatenate([coef_arr, xv, wv], axis=1, dtype=np.float32)
        )
        assert aux.shape == (128, AUX_COLS)
        in_maps.append({"xt": xt, "idx": idx_arr, "aux": aux})

    return in_maps, const0


# ---------------------------------------------------------------- entry
def run(inputs: dict, trace: bool = False):
    from concourse import bass_utils

    in_maps, const0 = prepare_inputs(
        inputs["x"], inputs["all_vars"], inputs["all_factors"]
    )
    nc, out_name = get_program()
    res = bass_utils.run_bass_kernel_spmd(
        nc, in_maps, core_ids=list(range(N_CORES)), trace=trace
    )
    partials = np.stack([np.asarray(r[out_name]).reshape(B) for r in res.results])
    ll = partials.astype(np.float64).sum(axis=0) + const0
    return ll.astype(np.float32), res


def kernel(x, binary_combinations, all_vars, all_factors):
    out, _ = run(
        {"x": x, "all_vars": all_vars, "all_factors": all_factors}
    )
    return out
